# revision 43
# baseline (speedup 1.0000x reference)
"""Trainium2 Bass kernel v2 for nn_LocalDynamics (GNN message passing).

Design: dest-shard across 8 cores; host bin-packs each core's 12500 local
nodes into W=100 windows x 128 slots (caps per window: lf<=256, lt<=256,
gb<=128 messages with gens confined to windows 0..95, <=128 rows; greedy +
single-swap augmentation since lf/lt fill is ~98%) so every stream has a
STATIC block->window map (B blocks of 128 tokens per window) identical on
all cores (SPMD-safe). W=100 is the minimum for B=2 (W*256 >= worst-core
line count, W divisible by 4), minimizing the 400 INDIRECT1D gather calls
that set the span.
Per 1024-token tile:
  - other-endpoint rows: per-block SWDGE indirect DMA from bf16 h in HBM
  - dest rows: PE selection matmul (Sel2row) against SBUF-resident window
    tiles of the (permuted) local h slice -> act partitions 64:128; the
    other half is PE-transposed into partitions 0:64 of the same PSUM block
  - 3-layer MLP in bf16 (tanh via ACT, h_global/t folded into bias1)
  - delta reduce: PE matmul with Sel2 (token->slot equality) accumulating
    each window's B blocks in PSUM, one DVE add per tile into SBUF delta
Final tanh(delta) -> DRAM; host un-permutes slots and concatenates cores.
"""
import os
import sys
import numpy as np

sys.path.insert(0, "/opt/trn_rl_repo")

from concourse import bass, bacc, mybir, tile
from concourse.bass_utils import run_bass_kernel_spmd
from concourse.masks import make_identity
import ml_dtypes

bf16 = ml_dtypes.bfloat16

N = 100000
NCORES = 8
SL = N // NCORES          # 12500 local nodes per core
W = 100                   # windows per core (128 slots each)
GBW = 96                  # gens confined to windows 0..95 (12 full tiles)
T = 1024                  # tokens per tile
NB = T // 128             # blocks per tile

CAPS = {"lf": 256, "lt": 256, "gb": 128}
BLK = {"lf": 2, "lt": 2, "gb": 1}        # blocks per window per stream
XD = {"lf": 8, "lt": 8, "gb": 4}
SW = {"lf": W, "lt": W, "gb": GBW}   # windows per stream
# HW INDIRECT1D only honors one-index-per-partition [128,1] offset APs
# (multi-index-per-partition batching reads garbage offsets on HW).
BATCH_GATHER = False
# windows whose 2nd lf/lt block is kept all-pad by the packer (cap 128
# instead of 256) so its gather call can be skipped entirely
SKIP_LF = set(range(96, 104))
SKIP_LT = set(range(88, 96))


def _pack_core(c, dests, others, xs, skip_caps=True):
    """Bin-pack rows into windows; build per-stream token arrays for core c.

    dests/others/xs: dict stream -> (local dest ids, global other ids or
    None, x rows) already masked to this core.
    With skip_caps, windows in SKIP_LF/SKIP_LT are capped at 128 lf/lt
    messages so their 2nd block stays all-pad (gather call skipped).
    Returns (slot[SL], per-stream dict of arrays).
    """
    deg = np.zeros((SL, 3), np.int64)
    for si, s in enumerate(("lf", "lt", "gb")):
        np.add.at(deg[:, si], dests[s], 1)
    caps = np.tile(np.array([CAPS["lf"], CAPS["lt"], CAPS["gb"], 128],
                            np.int64), (W, 1))
    caps[GBW:, 2] = 0        # gens confined to windows 0..GBW-1

    loads = np.zeros((W, 4), np.int64)
    binof = np.full(SL, -1, np.int64)
    nodes_in = [[] for _ in range(W)]
    order = np.argsort(-(deg.sum(1)), kind="stable")
    need = np.concatenate([deg, np.ones((SL, 1), np.int64)], 1)
    capsf = np.maximum(caps, 1).astype(np.float64)

    def _augment(r):
        # no window fits r directly: relocate one resident node r2 from
        # some window w (making room for r there) to a window w2
        ndr = need[r]
        for w in range(W):
            for r2 in nodes_in[w]:
                nd2 = need[r2]
                if not ((loads[w] - nd2 + ndr) <= caps[w]).all():
                    continue
                feas2 = (loads + nd2 <= caps).all(axis=1)
                feas2[w] = False
                if not feas2.any():
                    continue
                sc = (loads / capsf).max(axis=1)
                sc[~feas2] = np.inf
                w2 = int(np.argmin(sc))
                loads[w] -= nd2
                loads[w2] += nd2
                binof[r2] = w2
                nodes_in[w].remove(r2)
                nodes_in[w2].append(r2)
                return w
        raise RuntimeError("bin packing failed; raise W")

    for r in order:
        nd = need[r]
        feas = (loads + nd <= caps).all(axis=1)
        if feas.any():
            score = (loads / capsf).max(axis=1)
            score[~feas] = np.inf
            w = int(np.argmin(score))
        else:
            w = _augment(r)
        binof[r] = w
        loads[w] += nd
        nodes_in[w].append(r)

    # assign slot positions within each window
    slotin = np.zeros(SL, np.int64)
    fill = np.zeros(W, np.int64)
    for r in range(SL):
        w = binof[r]
        slotin[r] = fill[w]
        fill[w] += 1
    slot = binof * 128 + slotin

    out = {}
    for s in ("lf", "lt", "gb"):
        d = dests[s]
        B = BLK[s]
        S = SW[s] * B * 128
        sl = slot[d]
        wt = sl // 128
        jt = sl % 128
        o = np.argsort(wt, kind="stable")
        wt_s, jt_s = wt[o], jt[o]
        n = len(d)
        if n:
            starts = np.r_[True, wt_s[1:] != wt_s[:-1]]
            sidx = np.where(starts)[0]
            k = np.arange(n) - np.repeat(sidx, np.diff(np.r_[sidx, n]))
        else:
            k = np.zeros(0, np.int64)
        pos = wt_s * (B * 128) + k
        assert len(pos) == 0 or k.max() < B * 128

        oidx_arr = np.zeros(S, np.int32)
        if others[s] is not None:
            oidx_arr[pos] = others[s][o]
        dcol_arr = np.full(S, 999, np.int32)
        dcol_arr[pos] = jt_s
        x_arr = np.zeros((S, XD[s]), np.float32)
        x_arr[pos] = xs[s][o]

        # s2r[p, tok] = (p == slot_j(tok)): window-select rhs (slot-major)
        s2r = (np.arange(128, dtype=np.int32)[:, None]
               == dcol_arr[None, :]).astype(bf16)
        # sel[p, blk*128+j] = (slot_j(tok=(blk,p)) == j): reduce lhsT
        D = dcol_arr.reshape(S // 128, 128)          # [blk, p]
        sel = (D[:, :, None] == np.arange(128, dtype=np.int32)).astype(bf16)
        sel = np.ascontiguousarray(sel.transpose(1, 0, 2).reshape(128, S))

        out[s] = {
            "oidx": np.ascontiguousarray(
                oidx_arr.reshape(S // 128, 128).T).astype(np.int32),
            "s2r": np.ascontiguousarray(s2r),
            "sel": sel,
            "x": np.ascontiguousarray(x_arr.T).astype(bf16),
        }
    return slot, out


def _prep_weights(inputs):
    """Stream weight slices for act layout [other(0:64) | dest(64:128)]."""
    ws = {}
    for s in ("lf", "lt", "gb"):
        pre = s
        w1 = np.asarray(inputs[f"{pre}_w1"], np.float32)
        if s == "lf":       # dest=from rows[0:64], other=to rows[64:128]
            w1do = np.vstack([w1[64:128], w1[0:64]])
            w1x, w1gt = w1[128:136], w1[136:153]
        elif s == "lt":     # dest=to rows[64:128], other=from rows[0:64]
            w1do = np.vstack([w1[0:64], w1[64:128]])
            w1x, w1gt = w1[128:136], w1[136:153]
        else:               # gb: x rows[64:68] -> act rows 0:4; dest 64:128
            w1do = np.zeros((128, 128), np.float32)
            w1do[0:4] = w1[64:68]
            w1do[64:128] = w1[0:64]
            w1x, w1gt = None, w1[68:85]
        ws[s] = {
            "w1do": w1do.astype(bf16),
            "w1gt": w1gt.astype(np.float32),
            "b1": np.asarray(inputs[f"{pre}_b1"], np.float32).reshape(128, 1),
            "w2": np.asarray(inputs[f"{pre}_w2"], np.float32).astype(bf16),
            "b2": np.asarray(inputs[f"{pre}_b2"], np.float32).reshape(128, 1),
            "w3": np.asarray(inputs[f"{pre}_w3"], np.float32).astype(bf16),
            "b3": np.asarray(inputs[f"{pre}_b3"], np.float32).reshape(1, 64),
        }
        if w1x is not None:
            ws[s]["w1x"] = w1x.astype(bf16)
    return ws


def _build(nc, skip_pads=True):
    f32, i32, bfd = mybir.dt.float32, mybir.dt.int32, mybir.dt.bfloat16
    t_hbf = nc.dram_tensor("hbf", [N, 64], bfd, kind="ExternalInput")
    t_hsl = nc.dram_tensor("hsl", [128, W, 128], bfd, kind="ExternalInput")
    t_hgt = nc.dram_tensor("hgt", [17, 1], f32, kind="ExternalInput")
    t_out = nc.dram_tensor("out", [W * 128, 64], f32, kind="ExternalOutput")

    P = {}
    for s in ("lf", "lt", "gb"):
        S = SW[s] * BLK[s] * 128
        P[s] = {
            "s2r": nc.dram_tensor(f"{s}_s2r", [128, S], bfd,
                                  kind="ExternalInput"),
            "sel": nc.dram_tensor(f"{s}_sel", [128, S], bfd,
                                  kind="ExternalInput"),
            "x": nc.dram_tensor(f"{s}_x", [XD[s], S], bfd,
                                kind="ExternalInput"),
            "w1do": nc.dram_tensor(f"{s}_w1do", [128, 128], bfd,
                                   kind="ExternalInput"),
            "w1gt": nc.dram_tensor(f"{s}_w1gt", [17, 128], f32,
                                   kind="ExternalInput"),
            "b1": nc.dram_tensor(f"{s}_b1", [128, 1], f32,
                                 kind="ExternalInput"),
            "w2": nc.dram_tensor(f"{s}_w2", [128, 128], bfd,
                                 kind="ExternalInput"),
            "b2": nc.dram_tensor(f"{s}_b2", [128, 1], f32,
                                 kind="ExternalInput"),
            "w3": nc.dram_tensor(f"{s}_w3", [128, 64], bfd,
                                 kind="ExternalInput"),
            "b3": nc.dram_tensor(f"{s}_b3", [1, 64], f32,
                                 kind="ExternalInput"),
        }
        if s != "gb":
            P[s]["oidx"] = nc.dram_tensor(f"{s}_oidx", [128, S // 128], i32,
                                          kind="ExternalInput")
            P[s]["w1x"] = nc.dram_tensor(f"{s}_w1x", [8, 128], bfd,
                                         kind="ExternalInput")
    # flat per-tile gather indices: row r = tile (lf tiles then lt tiles),
    # position k = p*NB + b  (HW INDIRECT1D reads offsets contiguously from
    # the AP's first partition, ignoring partition strides)
    n_gtile = 2 * (W * BLK["lf"] * 128 // T)
    t_oidxf = nc.dram_tensor("oidxf", [n_gtile, T], i32,
                             kind="ExternalInput")

    with tile.TileContext(nc) as tc:
        with (
            tc.tile_pool(name="const", bufs=1) as cpool,
            tc.tile_pool(name="idx", bufs=1) as ipool,
            tc.tile_pool(name="work", bufs=4) as wpool,
            tc.tile_pool(name="ps_a", bufs=2, space="PSUM") as papool,
        ):
            # oidx first on the sync queue so gathers start immediately;
            # bulk hsl load goes via the scalar HWDGE queue instead
            oidx_first = {}
            for s in ("lf", "lt"):
                S = W * BLK[s] * 128
                oidx_t = ipool.tile([128, S // 128], i32, tag=f"{s}oidx")
                oidx_first[s] = oidx_t
                nc.sync.dma_start(oidx_t[:], P[s]["oidx"][:])

            ident = cpool.tile([128, 128], bfd)
            make_identity(nc, ident[:])
            ones1 = cpool.tile([1, 128], f32)
            nc.vector.memset(ones1[:], 1.0)
            hgt_sb = cpool.tile([17, 1], f32)
            nc.sync.dma_start(hgt_sb[:], t_hgt[:])
            hsl = cpool.tile([128, W, 128], bfd)
            nc.scalar.dma_start(hsl[:], t_hsl[:])

            # delta as one tile per 4-window output chunk: every lf/lt tile
            # it (and gb half) writes exactly one chunk tile, so the output
            # for chunk it can be emitted right after lt tile it with exact
            # whole-tile dependencies (slice-level tracking raced/NaN'd)
            dtiles = []
            for k in range(W // 4):
                dt = cpool.tile([128, 4, 64], f32, tag=f"delta{k}")
                nc.vector.memset(dt[:], 0.0)
                dtiles.append(dt)
            oview = t_out.ap().rearrange("(w p) d -> p w d", p=128)

            # ---- all other-endpoint gathers issued upfront (GpSimd is the
            # critical path at ~10ns/row; let it run flat-out from t=0 into
            # fully-resident g tiles, one INDIRECT1D per 1024-token tile).
            # HW reads the offsets contiguously from the AP's first
            # partition, so index rows are packed flat (k = p*NB + b). ----
            n_gtile = 2 * (W * BLK["lf"] * 128 // T)
            gtiles = {}
            if BATCH_GATHER:
                oidxf = ipool.tile([n_gtile, T], i32, tag="oidxf")
                nc.sync.dma_start(oidxf[:], t_oidxf[:])
                for s, base in (("lf", 0), ("lt", n_gtile // 2)):
                    for it in range(n_gtile // 2):
                        g = cpool.tile([128, NB, 64], bfd, tag=f"g_{s}_{it}")
                        gtiles[(s, it)] = g
                        row = base + it
                        nc.gpsimd.indirect_dma_start(
                            out=g[:], out_offset=None, in_=t_hbf[:],
                            in_offset=bass.IndirectOffsetOnAxis(
                                ap=oidxf[row:row + 1, :], axis=0))
            else:
                oidx_sb = oidx_first
                for s in ("lf", "lt"):
                    S, B = W * BLK[s] * 128, BLK[s]
                    skipw = (SKIP_LF if s == "lf" else SKIP_LT) \
                        if skip_pads else set()
                    for it in range(S // T):
                        g = cpool.tile([128, NB, 64], bfd, tag=f"g_{s}_{it}")
                        gtiles[(s, it)] = g
                        blks = range(it * NB, (it + 1) * NB)
                        skip = {c for c in blks
                                if c // B in skipw and c % B == B - 1}
                        if skip:
                            # zero the whole tile: skipped blocks must not
                            # feed NaN garbage through the (masked) MLP
                            nc.vector.memset(g[:], 0.0)
                        for b in range(NB):
                            c = it * NB + b
                            if c in skip:
                                continue
                            nc.gpsimd.indirect_dma_start(
                                out=g[:, b, :], out_offset=None, in_=t_hbf[:],
                                in_offset=bass.IndirectOffsetOnAxis(
                                    ap=oidx_sb[s][:, c:c + 1], axis=0))

            # gb first: its tiles need no gathers, so they hide under the
            # start of the (span-setting) gather stream
            for s in ("gb", "lf", "lt"):
                S, B, xd = SW[s] * BLK[s] * 128, BLK[s], XD[s]
                has_oth = s != "gb"
                pp = P[s]

                w1do = cpool.tile([128, 128], bfd, tag=f"{s}w1do")
                w1gt = cpool.tile([17, 128], f32, tag=f"{s}w1gt")
                b1t = cpool.tile([128, 1], f32, tag=f"{s}b1")
                w2 = cpool.tile([128, 128], bfd, tag=f"{s}w2")
                b2t = cpool.tile([128, 1], f32, tag=f"{s}b2")
                w3 = cpool.tile([128, 64], bfd, tag=f"{s}w3")
                b3t = cpool.tile([1, 64], f32, tag=f"{s}b3")
                loads = [(w1do, "w1do"), (w1gt, "w1gt"), (b1t, "b1"),
                         (w2, "w2"), (b2t, "b2"), (w3, "w3"), (b3t, "b3")]
                if has_oth:
                    w1x = cpool.tile([8, 128], bfd, tag=f"{s}w1x")
                    loads.append((w1x, "w1x"))
                for tl, pr in loads:
                    nc.sync.dma_start(tl[:], pp[pr][:])

                # bias1 = b1 + [hg;t] @ w1gt; b3 replicated to 128 partitions
                pb = papool.tile([128, 512], f32, tag="p1")
                nc.tensor.matmul(pb[:, 0:1], w1gt[:], hgt_sb[:],
                                 start=True, stop=True)
                bias1 = cpool.tile([128, 1], f32, tag=f"{s}bias1")
                nc.vector.tensor_tensor(out=bias1[:], in0=pb[:, 0:1],
                                        in1=b1t[:], op=mybir.AluOpType.add)
                pb2 = papool.tile([128, 512], f32, tag="p1")
                nc.tensor.matmul(pb2[:, 0:64], ones1[:], b3t[:],
                                 start=True, stop=True)
                b3rep = cpool.tile([128, 64], f32, tag=f"{s}b3rep")
                nc.vector.tensor_copy(b3rep[:], pb2[:, 0:64])

                ntile = S // T
                wpt = NB // B            # windows per tile
                hwin = 4 // B

                def emit_A(it):
                    # tile loads + act build for tile `it`, emitted one
                    # tile AHEAD of phases B/C: engine queues are strict
                    # in-order, so without this the PE head-of-line blocks
                    # on ACT results (h1/h2/m) with ~40% PE idle; hoisting
                    # the independent select/transpose matmuls fills those
                    # stalls.
                    s2r = wpool.tile([128, NB, 128], bfd, tag="s2r")
                    nc.sync.dma_start(
                        s2r[:].rearrange("p b j -> p (b j)"),
                        pp["s2r"][:, it * T:(it + 1) * T])
                    sel = wpool.tile([128, NB, 128], bfd, tag="sel")
                    nc.sync.dma_start(
                        sel[:].rearrange("p b j -> p (b j)"),
                        pp["sel"][:, it * T:(it + 1) * T])
                    xa = None
                    if has_oth:
                        xa = wpool.tile([8, T], bfd, tag="xa")
                        nc.sync.dma_start(xa[:], pp["x"][:, it * T:(it + 1) * T])
                        g = gtiles[(s, it)]

                    act = wpool.tile([128, T], bfd, tag="act")
                    for half in range(2):
                        hs = slice(half * 512, (half + 1) * 512)
                        b0 = half * 4
                        # act build: per-window select (B blocks share the
                        # stationary window tile) + transpose of gathered
                        # other rows into partitions 0:64
                        pacth = papool.tile([128, 4, 128], f32, tag="pact")
                        for wi in range(hwin):
                            wdx = it * wpt + half * hwin + wi
                            nc.tensor.matmul(
                                pacth[:, wi * B:(wi + 1) * B, :],
                                hsl[:, wdx, :],
                                s2r[:, b0 + wi * B:b0 + (wi + 1) * B, :],
                                start=True, stop=not has_oth)
                        if has_oth:
                            for bb in range(4):
                                nc.tensor.matmul(pacth[0:64, bb, :],
                                                 g[:, b0 + bb, :], ident[:],
                                                 start=False, stop=True)
                        nc.vector.tensor_copy(
                            act[:, hs].rearrange("p (b j) -> p b j", b=4),
                            pacth[:])
                        if not has_oth:
                            # x rows live in act partitions 0:xd
                            nc.sync.dma_start(
                                act[0:xd, hs],
                                pp["x"][:, it * T + half * 512:
                                        it * T + (half + 1) * 512])
                    return act, xa, sel

                def emit_BC(it, act, xa, sel):
                    h1 = wpool.tile([128, T], bfd, tag="h1")
                    h2 = wpool.tile([128, T], bfd, tag="h2")
                    m = wpool.tile([128, NB, 64], bfd, tag="m")
                    # phase B: pair same-stationary matmuls across halves so
                    # the redundant LDWEIGHTS can pull ahead into the
                    # background weight buffer and the MMs pipeline
                    hs0, hs1 = slice(0, 512), slice(512, 1024)
                    p1a = papool.tile([128, 512], f32, tag="p1")
                    p1b = papool.tile([128, 512], f32, tag="p1")
                    if has_oth:
                        nc.tensor.matmul(p1a[:], w1do[:], act[:, hs0],
                                         start=True, stop=False)
                        nc.tensor.matmul(p1b[:], w1do[:], act[:, hs1],
                                         start=True, stop=False)
                        nc.tensor.matmul(p1a[:], w1x[:], xa[:, hs0],
                                         start=False, stop=True)
                        nc.tensor.matmul(p1b[:], w1x[:], xa[:, hs1],
                                         start=False, stop=True)
                    else:
                        nc.tensor.matmul(p1a[:], w1do[:], act[:, hs0],
                                         start=True, stop=True)
                        nc.tensor.matmul(p1b[:], w1do[:], act[:, hs1],
                                         start=True, stop=True)
                    nc.scalar.activation(h1[:, hs0], p1a[:],
                                         mybir.ActivationFunctionType.Tanh,
                                         bias=bias1[:])
                    nc.scalar.activation(h1[:, hs1], p1b[:],
                                         mybir.ActivationFunctionType.Tanh,
                                         bias=bias1[:])
                    p2a = papool.tile([128, 512], f32, tag="p2")
                    p2b = papool.tile([128, 512], f32, tag="p2")
                    nc.tensor.matmul(p2a[:], w2[:], h1[:, hs0],
                                     start=True, stop=True)
                    nc.tensor.matmul(p2b[:], w2[:], h1[:, hs1],
                                     start=True, stop=True)
                    nc.scalar.activation(h2[:, hs0], p2a[:],
                                         mybir.ActivationFunctionType.Tanh,
                                         bias=b2t[:])
                    nc.scalar.activation(h2[:, hs1], p2b[:],
                                         mybir.ActivationFunctionType.Tanh,
                                         bias=b2t[:])

                    # phase C: both halves' w3 first (fills the m-tanh
                    # bubble before each half's pd matmuls)
                    p3pds = []
                    for half in range(2):
                        b0 = half * 4
                        # p3 (cols 0:256) and pd (cols 256:512) share a bank
                        p3pd = papool.tile([128, 512], f32, tag="p3pd")
                        p3pds.append(p3pd)
                        p3 = p3pd[:, 0:256].rearrange("p (b d) -> p b d", b=4)
                        for bb in range(4):
                            nc.tensor.matmul(
                                p3[:, bb, :],
                                h2[:, (b0 + bb) * 128:(b0 + bb + 1) * 128],
                                w3[:], start=True, stop=True)
                        nc.vector.tensor_tensor(
                            out=p3[:], in0=p3[:],
                            in1=b3rep[:].unsqueeze(1)
                            .to_broadcast([128, 4, 64]),
                            op=mybir.AluOpType.add)
                        nc.scalar.activation(m[:, b0:b0 + 4, :], p3[:],
                                             mybir.ActivationFunctionType.Tanh)

                    for half in range(2):
                        b0 = half * 4
                        pd = p3pds[half][:, 256:512].rearrange(
                            "p (b d) -> p b d", b=4)
                        for wi in range(hwin):
                            for bb in range(B):
                                b = b0 + wi * B + bb
                                nc.tensor.matmul(pd[:, wi, :], sel[:, b, :],
                                                 m[:, b, :], start=(bb == 0),
                                                 stop=(bb == B - 1))
                        w0 = it * wpt + half * hwin
                        dk, off = w0 // 4, w0 % 4
                        dview = dtiles[dk][:, off:off + hwin, :]
                        nc.vector.tensor_tensor(
                            out=dview, in0=dview,
                            in1=pd[:, 0:hwin, :],
                            op=mybir.AluOpType.add)

                    # lt runs last; lt tile it is the final writer of
                    # chunk tile it — emit its tanh+output immediately so
                    # the output phase hides under the gather stream
                    if s == "lt":
                        ft = wpool.tile([128, 4, 64], f32, tag="fout")
                        nc.scalar.activation(
                            ft[:], dtiles[it][:],
                            mybir.ActivationFunctionType.Tanh)
                        nc.sync.dma_start(oview[:, it * 4:(it + 1) * 4, :],
                                          ft[:])

                # software-pipeline driver: A runs one tile ahead of B/C
                nxt = emit_A(0)
                for it in range(ntile):
                    cur = nxt
                    nxt = emit_A(it + 1) if it + 1 < ntile else None
                    emit_BC(it, *cur)
    nc.compile()
    return nc


def kernel(**inputs):
    h_local = np.asarray(inputs["h_local"], np.float32)
    h_global = np.asarray(inputs["h_global"], np.float32).reshape(-1)
    x_line = np.asarray(inputs["x_line"], np.float32)
    x_gen = np.asarray(inputs["x_gen"], np.float32)
    tval = np.asarray(inputs["t"], np.float32).reshape(-1)
    line_from = np.asarray(inputs["line_from"], np.int64)
    line_to = np.asarray(inputs["line_to"], np.int64)
    gen_bus = np.asarray(inputs["gen_bus"], np.int64)

    h_bf = h_local.astype(bf16)
    hgt = np.concatenate([h_global, tval]).reshape(17, 1).astype(np.float32)
    ws = _prep_weights(inputs)
    per_core = []
    for c in range(NCORES):
        dests, others, xs = {}, {}, {}
        for s, (dest, oth, xr) in {
            "lf": (line_from, line_to, x_line),
            "lt": (line_to, line_from, x_line),
            "gb": (gen_bus, None, x_gen),
        }.items():
            mask = (dest // SL) == c
            dests[s] = (dest[mask] - c * SL).astype(np.int64)
            others[s] = oth[mask].astype(np.int32) if oth is not None else None
            xs[s] = xr[mask]
        per_core.append((dests, others, xs))

    # skip-capped packing (would let the kernel skip 16 gather calls) is
    # disabled: the greedy+repair packer can't reliably satisfy the
    # tightened caps on all cores
    skip_pads = False
    slots, packs = [], []
    for c in range(NCORES):
        slot, pk = _pack_core(c, *per_core[c], skip_caps=False)
        slots.append(slot)
        packs.append(pk)

    nc = bacc.Bacc("TRN2", target_bir_lowering=False, debug=False)
    nc = _build(nc, skip_pads=skip_pads)

    in_maps = []
    for c in range(NCORES):
        hsl = np.zeros((128, W, 128), np.float32)
        slot = slots[c]
        # hsl[j, w, 64:128] = h row of node with slot w*128+j
        wv, jv = slot // 128, slot % 128
        hsl[jv, wv, 64:128] = h_local[c * SL:(c + 1) * SL]
        m = {"hbf": h_bf, "hsl": hsl.astype(bf16), "hgt": hgt}
        for s in ("lf", "lt", "gb"):
            for k, v in packs[c][s].items():
                m[f"{s}_{k}"] = v
            for k, v in ws[s].items():
                m[f"{s}_{k}"] = v
        # flat gather-index rows: row = tile (lf then lt), pos k = p*NB + b
        ntile_s = W * BLK["lf"] * 128 // T
        oidxf = np.zeros((2 * ntile_s, T), np.int32)
        for si, s in enumerate(("lf", "lt")):
            ox = packs[c][s]["oidx"]              # [128, S//128]
            for it in range(ntile_s):
                oidxf[si * ntile_s + it] = \
                    ox[:, it * NB:(it + 1) * NB].reshape(-1)
        m["oidxf"] = np.ascontiguousarray(oidxf)
        in_maps.append(m)

    trace = bool(os.environ.get("BASS_KERNEL_TRACE"))
    res = run_bass_kernel_spmd(nc, in_maps, core_ids=list(range(NCORES)),
                               trace=trace)
    if trace and res.exec_time_ns:
        print(f"HW exec time: {res.exec_time_ns} ns")
    outs = []
    for c in range(NCORES):
        full = res.results[c]["out"]          # [W*128, 64]
        outs.append(full[slots[c]])
    return np.concatenate(outs, 0).astype(np.float32)

